# revision 10
# baseline (speedup 1.0000x reference)
"""DensePose FC head (4x fixed-offset deformable conv + relu) on 8 trn2 cores.

Sharding: pure data-parallel over the ROI dim N=128 -> 16 ROIs per core,
weights replicated. Each core runs the full 4-layer network out of SBUF:

  x1 = concat(features[16,512,28,28], maxpool4(fine_segm)[16,25,28,28])
  for w in w1..w4: x = relu(deform_conv_fixed(x, w))

The deformable conv with fixed integer taps is 9 shifted 1x1 GEMMs. Every tap
window is clipped to its valid output range (out-of-bounds contributions are
zero), so activation tiles are stored UNPADDED [128,28,28]; only the pooled
block xp keeps a 34x34 zero-padded plane for the packed-pool scatter.

v2 orchestration (vs v1):
- L1's 4 (hf,o) PSUM chains stay open across the layer so the pool matmuls
  run LAST: the pool pipeline (segm DMA -> DVE reduce -> scatter -> funnel)
  gets the whole L1 feature pass (~24us) of slack before PE consumes pk.
  Feature matmuls stay chunk-major (one bank at a time).
- relu runs exclusively on ScalarE: VectorE is left to the pool reduce, so
  the strict-FIFO DVE queue never head-of-line blocks a PSUM drain behind a
  reduce that waits on DMA.
- All DMAs on SP-HWDGE; ScalarE runs ONLY relus (a DMA trigger that waits
  on a ring WAR hazard in the strict-FIFO ACT queue would block the relus
  the PE needs for PSUM recycling - measured 35us priority inversions).
- w1 is packed in tap-consumption order and DMA'd in 8 chunks so the first
  matmul only waits for ~300KB; w2-4 DMAs are interleaved into the prologue
  produces (w2 must land before L2(0) at ~1.5 steps in).
- 3-ROI produce lead (rings of 4) and minimal memsets.
- v3: activation planes are [28, 32] with zero w-pads (3 left, 1 right;
  features pre-padded on the host). Every tap's output w-window is extended
  to even (8-byte) bounds - the extension columns multiply the zero pads, so
  they add exactly 0 - which makes every PSUM row write start AND end on an
  8-byte cacheline. Misaligned rows measured ~7-10ns/matmul extra; the
  extension costs one streaming cycle per row, net ~-15us.
"""

import numpy as np
import ml_dtypes
from contextlib import ExitStack

import concourse.bass as bass
import concourse.tile as tile
from concourse import bacc
from concourse import mybir
from concourse.bass_utils import run_bass_kernel_spmd

TAPS = [(-1, -3), (-1, -1), (-1, 1), (0, 0), (0, 2), (-2, 1), (0, -1), (2, 0), (3, 1)]
N_CORES = 8
N_FULL = 128          # total ROIs
NLOC = N_FULL // N_CORES
CIN, P1, HID = 512, 25, 256
R, PAD, PR = 28, 3, 34       # spatial, pad, padded spatial (xp only)
LP, PW = 3, 32               # activation planes [28, 32]: w-pads 3 left, 1 right
HALF = 14 * R                # 392 positions per chunk
HM = 112                     # fine_segm spatial
RING = 4
LEAD = 3

f32 = mybir.dt.float32
bf16 = mybir.dt.bfloat16

# tap (0,0) (full output coverage) first: it opens every chain with a full
# window so start=True zeroes all elements before partial-window taps land.
TAP_ORDER = [3, 0, 1, 2, 4, 5, 6, 7, 8]
TAP_POS = {t: i for i, t in enumerate(TAP_ORDER)}

# per-tap output w-window, extended to even (8-byte PSUM cacheline) bounds;
# the extension columns read the zero w-pads of the [28, 32] planes, so they
# contribute exactly 0 (see v3 note in the docstring)
TAP_W = {}
for _t, (_dh, _dw) in enumerate(TAPS):
    _lo, _hi = max(0, -_dw), min(R, R - _dw)
    _lo_e = _lo & ~1
    TAP_W[_t] = (_lo_e, _hi + ((_hi - _lo_e) & 1))


def _w_tile_idx(ktiles, t, k, o):
    # consumption order: taps in TAP_ORDER, k inner, o innermost
    return (TAP_POS[t] * ktiles + k) * 2 + o


# L1 pool-block cross-tap packing: the 25 pooled channels x 9 taps = 225 rows
# are packed into 2 dense k-tiles (rows (t, c) -> t*25+c), instead of paying a
# full 128-row matmul slot per tap for a 25-row contraction.
W1_NTILES = 9 * 4 * 2 + 4      # 72 feature tiles + 2 packed k-tiles x 2 ochunks
N_W_TILES = [W1_NTILES, 36, 36, 36]


def _pool_tap_segments(t):
    """Packed-row segments for tap t: list of (ptile, row_lo, c_lo, n_rows)."""
    g0, g1 = t * 25, t * 25 + 25
    segs = []
    for p in range(2):
        lo, hi = max(g0, p * 128), min(g1, (p + 1) * 128)
        if lo < hi:
            segs.append((p, lo - p * 128, lo - g0, hi - lo))
    return segs


def build_nc(nloc=NLOC, finalize=True):
    nc = bacc.Bacc()
    feats = nc.dram_tensor("features", [nloc, CIN, R * PW], bf16, kind="ExternalInput")
    segm = nc.dram_tensor("fine_segm", [nloc, P1 * HM * HM], bf16,
                          kind="ExternalInput")
    wts = [
        nc.dram_tensor(f"w{l + 1}p", [128, N_W_TILES[l], 128], bf16,
                       kind="ExternalInput")
        for l in range(4)
    ]
    out = nc.dram_tensor("out", [nloc, HID, R * R], f32, kind="ExternalOutput")

    with tile.TileContext(nc) as tc, ExitStack() as ctx:
        _body(ctx, tc, nloc, feats, segm, wts, out)
    if finalize:
        nc.finalize()
    return nc


def _body(ctx, tc, nloc, feats, segm, wts, out):
    nc = tc.nc
    singles = ctx.enter_context(tc.tile_pool(name="singles", bufs=1))
    sstage = ctx.enter_context(tc.tile_pool(name="sstage", bufs=3))
    pmid = ctx.enter_context(tc.tile_pool(name="pmid", bufs=3))
    pstage = ctx.enter_context(tc.tile_pool(name="pstage", bufs=3))
    ostage = ctx.enter_context(tc.tile_pool(name="ostage", bufs=8))
    psum = ctx.enter_context(tc.tile_pool(name="psum", bufs=8, space="PSUM"))

    # resident weights, one [128, T, 128] tile per layer. w1 lands in 8 chunks
    # (tiles are in consumption order, so the first chunk unblocks the first
    # matmuls); w2-4 DMAs are issued later, inside the ROI loop.
    w_sb = [singles.tile([128, N_W_TILES[l], 128], bf16, tag=f"w{l}", name=f"w{l}")
            for l in range(4)]

    def load_w(l, n_chunks, part=None):
        ntl = N_W_TILES[l]
        bounds = [ntl * i // n_chunks for i in range(n_chunks + 1)]
        for q in range(n_chunks) if part is None else part:
            lo, hi = bounds[q], bounds[q + 1]
            nc.sync.dma_start(out=w_sb[l][:, lo:hi, :], in_=wts[l][:, lo:hi, :])

    load_w(0, 8, part=range(1))

    # activation rings (unpadded): xf[4 feature blocks], y[l][2 out blocks];
    # xp (pooled block) keeps the 34x34 zero-padded plane for scatter reads
    def act_tile(tag, w=PW):
        return singles.tile([128, R, w], bf16, tag=tag, name=tag)

    xf = [[act_tile(f"xf{b}_{i}") for i in range(RING)] for b in range(4)]
    xp = [singles.tile([128, PR, PR], bf16, tag=f"xp_{i}", name=f"xp_{i}")
          for i in range(RING)]
    y = [[[act_tile(f"y{l}_{b}_{i}") for i in range(RING)] for b in range(2)]
         for l in range(3)]
    # cross-tap packed pooled-block rhs tiles (rows = (tap, channel))
    pk = [[act_tile(f"pk{p}_{i}", w=R) for i in range(RING)] for p in range(2)]

    # one-time zeroing: xp pads (only channels 0..24 are ever read) and the
    # pk[1] tail rows (never written by the funnel; their weights are zero,
    # but SBUF junk could be inf/nan)
    for t in xp:
        nc.gpsimd.memset(t[:P1, :, :], 0.0)
    for t in pk[1]:
        # engines base-address partitions at multiples of 32; 96 covers the
        # 97.. tail (rows 96 is rewritten by the funnel each ROI anyway)
        nc.gpsimd.memset(t[96:, :, :], 0.0)
    # y planes are relu-written on cols [3,31) only: zero each ring tile once
    # (after the pool-path memsets - ROI0's funnel needs those first - and
    # slot-major so slot 0 is ready when step 0's relus land)
    for i in range(RING):
        for l in range(3):
            for b in range(2):
                nc.gpsimd.memset(y[l][b][i][:, :, :], 0.0)

    def produce_l1_input(r, first=False):
        slot = r % RING
        # features block 0 first for ROI 0 (the very first matmul reads it),
        # then fine_segm (the pool path - reduce -> scatter -> funnel - is
        # the longest latency chain), then the remaining feature blocks.
        def feat_dma(b):
            nc.sync.dma_start(
                out=xf[b][slot][:, :, :],
                in_=feats[r, b * 128:(b + 1) * 128, :].rearrange(
                    "c (h w) -> c h w", w=PW))

        if first:
            feat_dma(0)
        sst = sstage.tile([128, 4 * 28 * 28], bf16, name="sst")
        segm_r = segm[r].rearrange("(p x) -> p x", x=3136)
        for b in range(4):
            nc.sync.dma_start(out=sst[25 * b:25 * (b + 1)],
                              in_=segm_r[25 * b:25 * (b + 1)])
        for b in range(0 if not first else 1, 4):
            feat_dma(b)
        pooled = pmid.tile([128, 196], bf16, name="pooled")
        nc.vector.tensor_reduce(
            out=pooled[:100],
            in_=sst[:100].rearrange("p (h0 h1 w0 w1) -> p h0 w0 h1 w1",
                                    h0=7, h1=4, w0=28, w1=4),
            axis=mybir.AxisListType.XY,
            op=mybir.AluOpType.max,
        )
        # [100, 196] -> [25, 28, 28] interior of the padded pool block
        dst = xp[slot][:P1, PAD:PAD + R, PAD:PAD + R].rearrange(
            "p (q h) w -> p q h w", q=4)
        nc.sync.dma_start(out=dst, in_=pooled[:100])
        # scatter 9 shifted copies of the pooled plane into packed STAGING
        # tiles (full 28x28 shifted windows read pad zeros), then one funnel
        # DMA per packed tile so the consuming matmuls wait on one queue.
        # All on SP-HWDGE: input loads (ACT-HWDGE) never sit behind these.
        pks = [pstage.tile([128, R, R], bf16, tag=f"pks{p}", name=f"pks{p}")
               for p in range(2)]
        for t, (dh, dw) in enumerate(TAPS):
            src_plane = xp[slot][:P1, PAD + dh:PAD + dh + R, PAD + dw:PAD + dw + R]
            for p, row_lo, c_lo, n in _pool_tap_segments(t):
                nc.sync.dma_start(
                    out=pks[p][row_lo:row_lo + n, :, :],
                    in_=src_plane[c_lo:c_lo + n],
                )
        nc.sync.dma_start(out=pk[0][slot][:, :, :], in_=pks[0][:, :, :])
        nc.sync.dma_start(out=pk[1][slot][:97, :, :], in_=pks[1][:97, :, :])

    def conv_layer(r, l):
        slot = r % RING
        if l == 0:
            xin = [xf[b][slot] for b in range(4)]
            ktiles, n_pool = 4, 2
        else:
            xin = [y[l - 1][b][slot] for b in range(2)]
            ktiles, n_pool = 2, 0
        # chunk-major feature matmuls: each (hf,o) chunk streams its whole
        # tap x k chain into ONE psum bank (no per-matmul bank alternation);
        # for L1 all 4 chains stay open so the pool matmuls can run LAST -
        # the pool pipeline (segm DMA -> DVE reduce -> scatter -> funnel)
        # gets the whole feature pass (~24us) of slack.
        ps = {}
        for hf in range(2):
            for o in range(2):
                ps[hf, o] = psum.tile([128, 14, R], f32, name="ps")
                for t in TAP_ORDER:
                    dh, dw = TAPS[t]
                    h_lo = max(max(0, -dh), 14 * hf)
                    h_hi = min(min(R, R - dh), 14 * hf + 14)
                    w_lo, w_hi = TAP_W[t]
                    for k in range(ktiles):
                        nc.tensor.matmul(
                            ps[hf, o][:, h_lo - 14 * hf:h_hi - 14 * hf,
                                      w_lo:w_hi],
                            lhsT=w_sb[l][:, _w_tile_idx(ktiles, t, k, o), :],
                            rhs=xin[k][:, h_lo + dh:h_hi + dh,
                                       w_lo + dw + LP:w_hi + dw + LP],
                            start=(t == TAP_ORDER[0] and k == 0),
                            stop=(n_pool == 0 and t == TAP_ORDER[-1]
                                  and k == ktiles - 1),
                        )
        for p in range(n_pool):
            for o in range(2):
                for hf in range(2):
                    nc.tensor.matmul(
                        ps[hf, o],
                        lhsT=w_sb[0][:, 72 + p * 2 + o, :],
                        rhs=pk[p][slot][:, hf * 14:hf * 14 + 14, :],
                        start=False,
                        stop=(p == n_pool - 1),
                    )
        # drain: relu on ScalarE only (VectorE stays free for the pool reduce)
        for hf in range(2):
            for o in range(2):
                if l < 3:
                    dst = y[l][o][slot][:, 14 * hf:14 * (hf + 1), LP:LP + R]
                    nc.scalar.activation(out=dst, in_=ps[hf, o],
                                         func=mybir.ActivationFunctionType.Relu)
                else:
                    ot = ostage.tile([128, 14, R], f32, name="ot")
                    nc.scalar.activation(out=ot, in_=ps[hf, o],
                                         func=mybir.ActivationFunctionType.Relu)
                    nc.sync.dma_start(
                        out=out[r, o * 128:(o + 1) * 128,
                                hf * HALF:(hf + 1) * HALF],
                        in_=ot.rearrange("p h w -> p (h w)"),
                    )

    # software-pipelined staircase over ROIs: at step s emit input-produce for
    # ROI s+LEAD and layer l for ROI s-l. Ring depth 4: xf[r%4] is written at
    # step r-3 and last read at step r.
    for r in range(min(LEAD, nloc)):
        produce_l1_input(r, first=(r == 0))
        if r == 0:
            load_w(0, 8, part=range(1, 8))
            load_w(1, 2)
        elif r == 1:
            load_w(2, 2)
        elif r == 2:
            load_w(3, 2)
    for s in range(nloc + 3):
        if s + LEAD < nloc:
            produce_l1_input(s + LEAD)
        for l in range(4):
            r = s - l
            if 0 <= r < nloc:
                conv_layer(r, l)


def _pack_weights(w, ktiles):
    """[O=256, Cin, 3, 3] fp32 -> [128, 9*ktiles*2, 128] bf16 lhsT tiles."""
    O, Cin = w.shape[:2]
    wf = w.reshape(O, Cin, 9)
    wp = np.zeros((128, 9 * ktiles * 2, 128), np.float32)
    for t in range(9):
        for k in range(ktiles):
            cs = min(128, Cin - k * 128)
            for o in range(2):
                ti = _w_tile_idx(ktiles, t, k, o)
                blk = wf[o * 128:(o + 1) * 128, k * 128:k * 128 + cs, t]
                wp[:cs, ti, :] = blk.T
    return wp.astype(ml_dtypes.bfloat16)


def _pack_w1(w):
    """w1 [256, 537, 3, 3] -> [128, 76, 128]: 72 feature lhsT tiles plus 4
    cross-tap-packed pooled-block tiles (packed row t*25+c <-> tap t, ch c)."""
    wf = w.reshape(HID, 537, 9)
    wp = np.zeros((128, W1_NTILES, 128), np.float32)
    for t in range(9):
        for k in range(4):
            for o in range(2):
                ti = _w_tile_idx(4, t, k, o)
                blk = wf[o * 128:(o + 1) * 128, k * 128:(k + 1) * 128, t]
                wp[:, ti, :] = blk.T
    for p in range(2):
        for o in range(2):
            ti = 72 + p * 2 + o
            for row in range(128):
                g = p * 128 + row
                if g >= 225:
                    break
                t, c = divmod(g, 25)
                wp[row, ti, :] = wf[o * 128:(o + 1) * 128, CIN + c, t]
    return wp.astype(ml_dtypes.bfloat16)


_CACHE = {}


def kernel(features, fine_segm, w1, w2, w3, w4):
    assert features.shape == (N_FULL, CIN, R, R), features.shape
    assert fine_segm.shape == (N_FULL, P1, HM, HM), fine_segm.shape

    if "nc" not in _CACHE:
        _CACHE["nc"] = build_nc(NLOC)
    nc = _CACHE["nc"]

    wpacked = {"w1p": _pack_w1(np.asarray(w1, np.float32))}
    for l, w in enumerate([w2, w3, w4], start=1):
        wpacked[f"w{l + 1}p"] = _pack_weights(np.asarray(w, np.float32), 2)
    # host-side bf16: feature values match what the on-chip fp32->bf16 copy
    # used to produce, and max-pooling bf16-rounded segm values equals
    # rounding the fp32 max (rounding is monotone) - results are identical
    fpad = np.zeros((N_FULL, CIN, R, PW), np.float32)
    fpad[:, :, :, LP:LP + R] = np.asarray(features, np.float32)
    featsr = np.ascontiguousarray(
        fpad.reshape(N_FULL, CIN, R * PW).astype(ml_dtypes.bfloat16))
    segmr = np.ascontiguousarray(
        np.asarray(fine_segm, np.float32).reshape(N_FULL, P1 * HM * HM)
        .astype(ml_dtypes.bfloat16))

    in_maps = []
    for c in range(N_CORES):
        sl = slice(c * NLOC, (c + 1) * NLOC)
        in_maps.append({"features": featsr[sl], "fine_segm": segmr[sl], **wpacked})

    res = run_bass_kernel_spmd(nc, in_maps, list(range(N_CORES)))
    outs = [res.results[c]["out"].reshape(NLOC, HID, R, R) for c in range(N_CORES)]
    return np.concatenate(outs, axis=0).astype(np.float32)
